# revision 23
# baseline (speedup 1.0000x reference)
"""CZ gate (wires i=0, j=11) on a batch of 22-qubit statevectors.

The CZ gate is diagonal: it negates amplitude idx iff bit(n-1-i) and
bit(n-1-j) of idx are both 1.  For n=22, i=0, j=11 that is bit 21 and
bit 10.  Viewing each statevector as 4096 rows of 1024 floats, row r is
negated iff r >= 2048 (bit 21) and r is odd (bit 10 = LSB of r).

Sharding: pure data parallel — batch 8 across 8 NeuronCores, one full
statevector (16 MiB f32) per core.  Per core:
  - first half (8 MiB, untouched by the gate): DRAM->DRAM DMA copy,
    split across both HWDGE rings to keep them co-busy
  - second half (8 MiB): streamed through SBUF in contiguous chunks
    (consecutive rows per partition), odd-row slices negated in-place
    on VectorE, stored back contiguously
"""

import sys

for _p in ("/opt/trn_rl_repo",):
    if _p not in sys.path:
        sys.path.insert(0, _p)

import numpy as np

import concourse.bass as bass
import concourse.mybir as mybir
from concourse.bass_utils import run_bass_kernel_spmd

NQUBIT = 22
N = 1 << NQUBIT          # 4194304 floats per statevector
BATCH = 8
N_CORES = 8
ROW = 1024               # floats per "row" (= 2^10, set by j=11 -> bit 10)
HALF = N // 2

# Set by test harness to capture a profile; results land in LAST_RESULT.
TRACE = False
LAST_RESULT = None

_NC_CACHE = {}


def _build_nc(nchunk=4, sp_copy=2**20, act_early=384 * 1024, pool_copy=0):
    """Raw-Bass kernel (no Tile): manual semaphores keep every instruction
    at <=1 sync wait (this walrus build rejects multi-wait instructions),
    and there is no Tile drain/barrier epilogue overhead.

    All DMAs are DRAM-contiguous (strided DRAM access measured ~82 GB/s
    vs ~400+ GB/s contiguous).  The whole second half streams through
    SBUF with consecutive rows per partition; only the odd-row slices
    (free-dim stride 2*ROW) are negated in-place on VectorE.

    The 8 MiB first-half DRAM->DRAM copy is split three ways to keep
    both HWDGE rings busy end-to-end (the SDMA arbiter serves rings at
    ~equal payload rate; a lone tail stream only reaches ~320 GB/s):
      SP  (sync):   ld0..ld{n}  (loads gate the whole chain - first),
                    then first-half copy piece A (sp_copy floats)
      ACT (scalar): copy piece B (act_early floats - fills the gap while
                    the first load lands), st0..st{n} as negates finish,
                    then copy piece C (remainder) to co-terminate with SP
      DVE (vector): in-place negate of odd-row slices
    """
    nc = bass.Bass()
    x = nc.dram_tensor("x", [N], mybir.dt.float32, kind="ExternalInput")
    y = nc.dram_tensor("y", [N], mybir.dt.float32, kind="ExternalOutput")

    chunk = HALF // nchunk               # floats per load/store chunk
    m_rows = chunk // (128 * ROW)        # rows per partition
    assert m_rows % 2 == 0 and m_rows * 128 * ROW == chunk
    shape = [128, m_rows, ROW]
    c0 = sp_copy                         # piece A: [0, c0) on SP
    c1 = c0 + act_early                  # piece B: [c0, c1) early on ACT
    c2 = c1 + pool_copy                  # piece P: [c1, c2) on GpSimd (SWDGE)

    import contextlib

    with contextlib.ExitStack() as ctx:
        tiles = [
            ctx.enter_context(nc.sbuf_tensor(f"t{g}", shape, mybir.dt.float32))
            for g in range(nchunk)
        ]
        # One semaphore per DMA: a single cumulative sem is racy - a fast
        # SDMA engine posts increments for DMA g+1 while a slow engine is
        # still moving DMA g, so cumulative counts over-report completion.
        lds = [ctx.enter_context(nc.semaphore(f"ld{g}")) for g in range(nchunk)]
        sts = [ctx.enter_context(nc.semaphore(f"st{g}")) for g in range(nchunk)]
        cps = [ctx.enter_context(nc.semaphore(f"cp{k}")) for k in range(4)]
        ve = ctx.enter_context(nc.semaphore("ve"))
        block = ctx.enter_context(nc.Block())

        def dram3(t, g):
            sl = t[HALF + g * chunk : HALF + (g + 1) * chunk]
            return sl.rearrange("(p m c) -> p m c", p=128, c=ROW)

        @block.sync
        def _(sync):
            for g in range(nchunk):
                sync.dma_start(tiles[g][:], dram3(x, g)).then_inc(lds[g], 16)
            sync.dma_start(y[0:c0], x[0:c0]).then_inc(cps[0], 16)
            for g in range(nchunk):
                sync.wait_ge(lds[g], 16)
            sync.wait_ge(cps[0], 16)

        @block.vector
        def _(vector):
            for g in range(nchunk):
                vector.wait_ge(lds[g], 16)
                # odd rows within each partition: m = 1, 3, 5, ...
                odd = tiles[g][:, 1::2, :]
                vector.tensor_scalar_mul(odd, odd, -1.0).then_inc(ve, 1)

        @block.scalar
        def _(scalar):
            scalar.dma_start(y[c0:c1], x[c0:c1]).then_inc(cps[1], 16)
            for g in range(nchunk):
                scalar.wait_ge(ve, g + 1)
                scalar.dma_start(dram3(y, g), tiles[g][:]).then_inc(sts[g], 16)
            scalar.dma_start(y[c2:HALF], x[c2:HALF]).then_inc(cps[2], 16)
            for g in range(nchunk):
                scalar.wait_ge(sts[g], 16)
            scalar.wait_ge(cps[1], 16)
            scalar.wait_ge(cps[2], 16)

        if pool_copy:

            @block.gpsimd
            def _(gpsimd):
                gpsimd.dma_start(y[c1:c2], x[c1:c2]).then_inc(cps[3], 16)
                gpsimd.wait_ge(cps[3], 16)

    return nc


def _numpy_fallback(x, i, j):
    n = int(round(np.log2(x.shape[1])))
    idx = np.arange(x.shape[1])
    mask = (((idx >> (n - 1 - i)) & 1) & ((idx >> (n - 1 - j)) & 1)).astype(bool)
    y = x.copy()
    y[:, mask] *= -1
    return y


def kernel(x, i, j):
    global LAST_RESULT
    x = np.ascontiguousarray(np.asarray(x, dtype=np.float32))
    i = int(np.asarray(i))
    j = int(np.asarray(j))
    if (i, j) != (0, 11) or x.shape != (BATCH, N):
        return _numpy_fallback(x, i, j)

    key = ("v1", TRACE)
    if key not in _NC_CACHE:
        _NC_CACHE[key] = _build_nc()
    nc = _NC_CACHE[key]

    in_maps = [{"x": x[c]} for c in range(N_CORES)]
    res = run_bass_kernel_spmd(
        nc, in_maps, core_ids=list(range(N_CORES)), trace=TRACE
    )
    LAST_RESULT = res
    return np.stack([r["y"] for r in res.results], axis=0)


# revision 25
# speedup vs baseline: 1.1429x; 1.1429x over previous
"""CZ gate (wires i=0, j=11) on a batch of 22-qubit statevectors.

The CZ gate is diagonal: it negates amplitude idx iff bit(n-1-i) and
bit(n-1-j) of idx are both 1.  For n=22, i=0, j=11 that is bit 21 and
bit 10.  Viewing each statevector as 4096 rows of 1024 floats, row r is
negated iff r >= 2048 (bit 21) and r is odd (bit 10 = LSB of r).

Sharding: pure data parallel — batch 8 across 8 NeuronCores, one full
statevector (16 MiB f32) per core.  Per core:
  - first half (8 MiB, untouched by the gate): DRAM->DRAM DMA copy,
    split across both HWDGE rings to keep them co-busy
  - second half (8 MiB): streamed through SBUF in contiguous chunks
    (consecutive rows per partition), odd-row slices negated in-place
    on VectorE, stored back contiguously
"""

import sys

for _p in ("/opt/trn_rl_repo",):
    if _p not in sys.path:
        sys.path.insert(0, _p)

import numpy as np

import concourse.bass as bass
import concourse.mybir as mybir
from concourse.bass_utils import run_bass_kernel_spmd

NQUBIT = 22
N = 1 << NQUBIT          # 4194304 floats per statevector
BATCH = 8
N_CORES = 8
ROW = 1024               # floats per "row" (= 2^10, set by j=11 -> bit 10)
HALF = N // 2

# Set by test harness to capture a profile; results land in LAST_RESULT.
TRACE = False
LAST_RESULT = None

_NC_CACHE = {}


def _build_nc(nchunk=4, sp_copy=1310720, act_early=0, pool_copy=0):
    """Raw-Bass kernel (no Tile): manual semaphores keep every instruction
    at <=1 sync wait (this walrus build rejects multi-wait instructions),
    and there is no Tile drain/barrier epilogue overhead.

    All DMAs are DRAM-contiguous (strided DRAM access measured ~82 GB/s
    vs ~400+ GB/s contiguous).  The whole second half streams through
    SBUF with consecutive rows per partition; only the odd-row slices
    (free-dim stride 2*ROW) are negated in-place on VectorE.

    The 8 MiB first-half DRAM->DRAM copy is split three ways to keep
    both HWDGE rings busy end-to-end (the SDMA arbiter serves rings at
    ~equal payload rate; a lone tail stream only reaches ~320 GB/s):
      SP  (sync):   ld0..ld{n}  (loads gate the whole chain - first),
                    then first-half copy piece A (sp_copy floats)
      ACT (scalar): copy piece B (act_early floats - fills the gap while
                    the first load lands), st0..st{n} as negates finish,
                    then copy piece C (remainder) to co-terminate with SP
      DVE (vector): in-place negate of odd-row slices
    """
    nc = bass.Bass()
    x = nc.dram_tensor("x", [N], mybir.dt.float32, kind="ExternalInput")
    y = nc.dram_tensor("y", [N], mybir.dt.float32, kind="ExternalOutput")

    chunk = HALF // nchunk               # floats per load/store chunk
    m_rows = chunk // (128 * ROW)        # rows per partition
    assert m_rows % 2 == 0 and m_rows * 128 * ROW == chunk
    shape = [128, m_rows, ROW]
    c0 = sp_copy                         # piece A: [0, c0) on SP
    c1 = c0 + act_early                  # piece B: [c0, c1) early on ACT
    c2 = c1 + pool_copy                  # piece P: [c1, c2) on GpSimd (SWDGE)

    import contextlib

    with contextlib.ExitStack() as ctx:
        tiles = [
            ctx.enter_context(nc.sbuf_tensor(f"t{g}", shape, mybir.dt.float32))
            for g in range(nchunk)
        ]
        # One semaphore per DMA: a single cumulative sem is racy - a fast
        # SDMA engine posts increments for DMA g+1 while a slow engine is
        # still moving DMA g, so cumulative counts over-report completion.
        lds = [ctx.enter_context(nc.semaphore(f"ld{g}")) for g in range(nchunk)]
        sts = [ctx.enter_context(nc.semaphore(f"st{g}")) for g in range(nchunk)]
        cps = [ctx.enter_context(nc.semaphore(f"cp{k}")) for k in range(4)]
        ve = ctx.enter_context(nc.semaphore("ve"))
        block = ctx.enter_context(nc.Block())

        def dram3(t, g):
            sl = t[HALF + g * chunk : HALF + (g + 1) * chunk]
            return sl.rearrange("(p m c) -> p m c", p=128, c=ROW)

        @block.sync
        def _(sync):
            for g in range(nchunk):
                sync.dma_start(tiles[g][:], dram3(x, g)).then_inc(lds[g], 16)
            sync.dma_start(y[0:c0], x[0:c0]).then_inc(cps[0], 16)
            for g in range(nchunk):
                sync.wait_ge(lds[g], 16)
            sync.wait_ge(cps[0], 16)

        @block.vector
        def _(vector):
            for g in range(nchunk):
                vector.wait_ge(lds[g], 16)
                # odd rows within each partition: m = 1, 3, 5, ...
                odd = tiles[g][:, 1::2, :]
                vector.tensor_scalar_mul(odd, odd, -1.0).then_inc(ve, 1)

        @block.scalar
        def _(scalar):
            if act_early:
                scalar.dma_start(y[c0:c1], x[c0:c1]).then_inc(cps[1], 16)
            for g in range(nchunk):
                scalar.wait_ge(ve, g + 1)
                scalar.dma_start(dram3(y, g), tiles[g][:]).then_inc(sts[g], 16)
            scalar.dma_start(y[c2:HALF], x[c2:HALF]).then_inc(cps[2], 16)
            for g in range(nchunk):
                scalar.wait_ge(sts[g], 16)
            if act_early:
                scalar.wait_ge(cps[1], 16)
            scalar.wait_ge(cps[2], 16)

        if pool_copy:

            @block.gpsimd
            def _(gpsimd):
                gpsimd.dma_start(y[c1:c2], x[c1:c2]).then_inc(cps[3], 16)
                gpsimd.wait_ge(cps[3], 16)

    return nc


def _numpy_fallback(x, i, j):
    n = int(round(np.log2(x.shape[1])))
    idx = np.arange(x.shape[1])
    mask = (((idx >> (n - 1 - i)) & 1) & ((idx >> (n - 1 - j)) & 1)).astype(bool)
    y = x.copy()
    y[:, mask] *= -1
    return y


def kernel(x, i, j):
    global LAST_RESULT
    x = np.ascontiguousarray(np.asarray(x, dtype=np.float32))
    i = int(np.asarray(i))
    j = int(np.asarray(j))
    if (i, j) != (0, 11) or x.shape != (BATCH, N):
        return _numpy_fallback(x, i, j)

    key = ("v1", TRACE)
    if key not in _NC_CACHE:
        _NC_CACHE[key] = _build_nc()
    nc = _NC_CACHE[key]

    in_maps = [{"x": x[c]} for c in range(N_CORES)]
    res = run_bass_kernel_spmd(
        nc, in_maps, core_ids=list(range(N_CORES)), trace=TRACE
    )
    LAST_RESULT = res
    return np.stack([r["y"] for r in res.results], axis=0)
